# revision 15
# baseline (speedup 1.0000x reference)
"""Multi-head attention (16 heads, D=128) on 8 trn2 NeuronCores.

Sharding: tensor-parallel over heads — each core owns 2 heads.
Per core: qkv projection for its 768 channels (chan-major for q/k,
token-major for v), fused RMSNorm+RoPE on q/k, SDPA in transposed-score
layout, partial proj over its 256 channels.  Host sums the 8 partial
outputs + bias.

Matmul operands are fp16; accumulation fp32 in PSUM.
exp is computed as exp(s/sqrt(D) - 4) — softmax-invariant shift that
keeps fp16 exp values in range.

Softmax denominator: exp tiles are summed across the 16 k-tiles on the
DVE (serial in-place accumulate), contracted over partitions by a single
[128,1]-ones matmul, inverted via Ln/Exp on [1,512], and broadcast to
all 128 partitions on the (otherwise idle) Pool engine — this keeps the
PE free of the 16-per-chunk ones-matmuls the previous version used.
The RMSNorm 1/rms row broadcast uses the same Pool path.

Cross-stage software pipeline: the per-engine queues execute in order,
so PE-heavy filler work (next batch's QKV chunks, previous batch's proj
tiles) is interleaved INSIDE each SDPA q-chunk's kt loop.  This keeps
the PE issue stream dense while the ACT engine works through the exp
cadence, instead of the PE head-blocking on a score bank whose exp
hasn't drained.
"""
import math
from contextlib import ExitStack

import numpy as np

import concourse.bass as bass
import concourse.mybir as mybir
import concourse.tile as tile
from concourse import bacc, bass_utils

F32 = mybir.dt.float32
F16 = mybir.dt.float16

H, D, B, N, C = 16, 128, 2, 2048, 2048
NCORES = 8
HPC = H // NCORES            # heads per core = 2
TOK = B * N                  # 4096
EPS = float(np.finfo(np.float32).eps)
SCALE = 1.0 / math.sqrt(D)
ESHIFT = -4.0                # exp(s*SCALE + ESHIFT); softmax-invariant

_CACHE = {}
RUN_KW = {}   # test.py sets {"trace": True}


def _pin_act_table():
    """Restrict Exp/Ln to the combined natural_log_exp_and_others set so the
    table-load pass keeps a single ACT table resident."""
    import concourse.hw_specs as hw
    tabs = hw.get_activation_tables("gen3")
    for name, funcs in tabs.items():
        if name != "natural_log_exp_and_others":
            funcs.discard(mybir.ActivationFunctionType.Exp)
            funcs.discard(mybir.ActivationFunctionType.Ln)


def build_module():
    """Build + compile the per-core Bass module (same NEFF for all cores)."""
    if "nc" in _CACHE:
        return _CACHE["nc"]
    _pin_act_table()
    nc = bacc.Bacc("TRN2", target_bir_lowering=False, debug=False,
                   num_devices=NCORES)

    xt_h = nc.dram_tensor("xt", [C, TOK], F16, kind="ExternalInput")
    wqk_h = nc.dram_tensor("wqk", [C, 4 * 128], F16, kind="ExternalInput")
    wv_h = nc.dram_tensor("wv", [C, 2 * 128], F16, kind="ExternalInput")
    wp_h = nc.dram_tensor("wp", [2 * 128, C], F16, kind="ExternalInput")
    cos2_h = nc.dram_tensor("cos2", [128, N], F16, kind="ExternalInput")
    sin2_h = nc.dram_tensor("sin2", [128, N], F16, kind="ExternalInput")
    qkb_h = nc.dram_tensor("qkb", [128, 4], F32, kind="ExternalInput")
    vb_h = nc.dram_tensor("vb", [128, 256], F32, kind="ExternalInput")
    invg2_h = nc.dram_tensor("invg2", [128, 2], F16, kind="ExternalInput")
    onecol_h = nc.dram_tensor("onecol", [128, 1], F16, kind="ExternalInput")
    eps_h = nc.dram_tensor("eps", [1, 1], F32, kind="ExternalInput")
    nb4_h = nc.dram_tensor("nb4", [128, 1], F32, kind="ExternalInput")
    y_h = nc.dram_tensor("y", [TOK, C], F16, kind="ExternalOutput")

    with tile.TileContext(nc) as tc, ExitStack() as ctx:
        pc = ctx.enter_context(tc.tile_pool(name="consts", bufs=1))
        p_xt = ctx.enter_context(tc.tile_pool(name="xt", bufs=4))
        p_qkv = ctx.enter_context(tc.tile_pool(name="qkv", bufs=2))
        p_qraw = ctx.enter_context(tc.tile_pool(name="qraw", bufs=2))
        p_qsw = ctx.enter_context(tc.tile_pool(name="qsw", bufs=3))
        p_sq = ctx.enter_context(tc.tile_pool(name="sq", bufs=2))
        p_exp = ctx.enter_context(tc.tile_pool(name="exp", bufs=2))
        p_acc = ctx.enter_context(tc.tile_pool(name="acc", bufs=2))
        p_bcast = ctx.enter_context(tc.tile_pool(name="bcast", bufs=2))
        p_ao = ctx.enter_context(tc.tile_pool(name="ao", bufs=2))
        p_y = ctx.enter_context(tc.tile_pool(name="y", bufs=6))
        p_row = ctx.enter_context(tc.tile_pool(name="rows", bufs=2))
        # PSUM: qk(2) + s(2) + av(2) + row(2) = 8 banks
        p_ps = ctx.enter_context(tc.tile_pool(name="ps", bufs=8, space="PSUM"))

        # ---- constants / weights ----
        # Startup critical path: the sync ring carries only wqk half-0 and the
        # first xt tiles (what the first matmuls wait on); everything else
        # rides the ACT engine's DMA ring, which is idle at startup.
        wqk = pc.tile([128, 16, 512], F16)
        wv = pc.tile([128, 16, 256], F16)
        nc.sync.dma_start(wqk[:, 0:8, :],
                          wqk_h.ap()[0:1024].rearrange("(t p) j -> p t j", p=128))
        nc.scalar.dma_start(wqk[:, 8:16, :],
                            wqk_h.ap()[1024:2048].rearrange("(t p) j -> p t j", p=128))
        qkb = pc.tile([128, 4], F32)
        nc.scalar.dma_start(qkb[:], qkb_h.ap())
        vb = pc.tile([128, 256], F32)
        nc.scalar.dma_start(vb[:], vb_h.ap())
        invg2 = pc.tile([128, 2], F16)
        nc.scalar.dma_start(invg2[:], invg2_h.ap())
        eps_t = pc.tile([1, 1], F32)
        nc.scalar.dma_start(eps_t[:], eps_h.ap())
        onecol = pc.tile([128, 1], F16)
        nc.scalar.dma_start(onecol[:], onecol_h.ap())
        nb4 = pc.tile([128, 1], F32)
        nc.scalar.dma_start(nb4[:], nb4_h.ap())

        cos2 = pc.tile([128, N], F16)
        sin2 = pc.tile([128, N], F16)
        wp = pc.tile([128, 2, 2048], F16)

        def load_late_consts():
            # emitted after the first chunk's xt DMAs; ACT ring, off the
            # startup critical path
            for hf in range(2):
                nc.scalar.dma_start(wv[:, hf * 8:(hf + 1) * 8, :],
                                    wv_h.ap()[hf * 1024:(hf + 1) * 1024]
                                    .rearrange("(t p) j -> p t j", p=128))
            nc.scalar.dma_start(cos2[:], cos2_h.ap())
            nc.scalar.dma_start(sin2[:], sin2_h.ap())
            nc.scalar.dma_start(wp[:], wp_h.ap().rearrange("(t p) j -> p t j", p=128))

        # ---------------- per-batch state ----------------
        state = {}

        def ph1(b, qraw_g, g0, gi):
            """sumsq + 1/rms rows for one 512-token group (4 chan-tiles)."""
            rrows = []
            for ct in range(4):
                is_k = ct // 2
                src_q = qraw_g[:, ct, :]
                sq = p_sq.tile([128, 512], F16, tag="sq", name=f"sq{b}{gi}{ct}")
                nc.vector.tensor_mul(out=sq[:], in0=src_q, in1=src_q)
                ps_ss = p_ps.tile([1, 512], F32, tag="row", bufs=2,
                                  name=f"ss{b}{gi}{ct}")
                nc.tensor.matmul(ps_ss[:], invg2[:, is_k:is_k + 1], sq[:],
                                 start=True, stop=True)
                # rrow = 1/sqrt(var+eps) = exp(-0.5*ln(var+eps))
                lrow = p_row.tile([1, 512], F32, tag="lrow", name=f"lr{b}{gi}{ct}")
                nc.scalar.activation(lrow[:], ps_ss[:],
                                     mybir.ActivationFunctionType.Ln,
                                     bias=eps_t[:], scale=1.0 / D)
                rrow = p_row.tile([1, 512], F16, tag="recip", bufs=4,
                                  name=f"rr{b}{gi}{ct}")
                nc.scalar.activation(rrow[:], lrow[:],
                                     mybir.ActivationFunctionType.Exp,
                                     scale=-0.5)
                rrows.append(rrow)
            return rrows

        def ph2(b, qraw_g, g0, gi, rrows):
            """RoPE + rms-scale for one 512-token group."""
            st = state[b]
            for ct in range(4):
                hl, is_k = ct % 2, ct // 2
                dst = (st["kT"] if is_k else st["qT"])
                src_q = qraw_g[:, ct, :]
                rsf = p_bcast.tile([128, 512], F16, tag="rsf",
                                   name=f"rsf{b}{gi}{ct}")
                nc.gpsimd.partition_broadcast(rsf[:], rrows[ct][:])
                qsw = p_qsw.tile([128, 512], F16, tag="qsw", name=f"qsw{b}{gi}{ct}")
                nc.sync.dma_start(qsw[0:64, :], src_q[64:128, :])
                nc.sync.dma_start(qsw[64:128, :], src_q[0:64, :])
                # in-place: qc into qraw, qs into qsw
                nc.vector.tensor_mul(out=src_q, in0=src_q,
                                     in1=cos2[:, g0:g0 + 512])
                nc.vector.tensor_mul(out=qsw[:], in0=qsw[:],
                                     in1=sin2[:, g0:g0 + 512])
                rot = dst[:, hl, g0:g0 + 512]
                nc.vector.tensor_add(out=rot, in0=src_q, in1=qsw[:])
                nc.vector.tensor_mul(out=rot, in0=rot, in1=rsf[:])

        def a_begin(b):
            st = state[b] = {}
            st["qT"] = p_qkv.tile([128, HPC, N], F16, tag="qT", name=f"qT{b}")
            st["kT"] = p_qkv.tile([128, HPC, N], F16, tag="kT", name=f"kT{b}")
            st["vtok"] = p_qkv.tile([128, 16, 256], F16, tag="v", name=f"v{b}")
            st["qraw"] = None
            st["ph1_pend"] = []
            st["ph2_pend"] = []

        def a_chunk_groups(b, ch):
            """QKV production for one 256-token chunk, as a generator that
            yields after each ~1.7us matmul group (6 groups per chunk)."""
            st = state[b]
            tok0 = b * N + ch * 256
            if ch % 2 == 0:
                st["qraw"] = p_qraw.tile([128, 4, 512], F16, tag="qraw",
                                         name=f"qraw{b}{ch}")
            qraw = st["qraw"]
            off = (ch % 2) * 256
            xts = []
            for half in range(2):
                xt = p_xt.tile([128, 8, 256], F16, tag="xt",
                               name=f"xt{b}{ch}{half}")
                src = xt_h.ap()[half * 1024:(half + 1) * 1024, tok0:tok0 + 256]
                nc.sync.dma_start(xt[:], src.rearrange("(t p) j -> p t j", p=128))
                xts.append(xt)
            if b == 0 and ch == 0:
                load_late_consts()
            # q/k chan-tiles, one PSUM bank each, sequential over ct
            for ct in range(4):
                ps_qk = p_ps.tile([128, 256], F32, tag="qk", bufs=2,
                                  name=f"a{b}{ch}{ct}")
                for half in range(2):
                    for kt in range(8):
                        nc.tensor.matmul(
                            ps_qk[:], wqk[:, half * 8 + kt, ct * 128:(ct + 1) * 128],
                            xts[half][:, kt, :],
                            start=(half == 0 and kt == 0),
                            stop=(half == 1 and kt == 7))
                nc.vector.tensor_scalar_add(qraw[:, ct, off:off + 256],
                                            ps_qk[:], qkb[:, ct:ct + 1])
                # inject deferred norm work so its ACT/PE latency hides
                if ct == 1 and st["ph1_pend"]:
                    args = st["ph1_pend"].pop()
                    st["ph2_pend"].append((args[0], args[1], args[2],
                                           ph1(b, *args)))
                elif ct == 3 and st["ph2_pend"]:
                    ph2(b, *st["ph2_pend"].pop())
                yield
            for s in range(2):
                ps_v = p_ps.tile([128, 256], F32, tag="qk", bufs=2,
                                 name=f"av{b}{ch}{s}")
                for half in range(2):
                    for kt in range(8):
                        nc.tensor.matmul(
                            ps_v[:], xts[half][:, kt, s * 128:(s + 1) * 128],
                            wv[:, half * 8 + kt, :],
                            start=(half == 0 and kt == 0),
                            stop=(half == 1 and kt == 7))
                nc.vector.tensor_add(out=st["vtok"][:, ch * 2 + s, :],
                                     in0=ps_v[:], in1=vb[:])
                yield
            if ch % 2 == 1:
                st["ph1_pend"].append((qraw, (ch - 1) * 256, ch // 2))
            if ch == 7:
                # flush the last group's norm epilogue
                args = st["ph1_pend"].pop()
                ph2(b, args[0], args[1], args[2], ph1(b, *args))

        # ---------------- proj ----------------
        def c_quarters(b, tt, ring=None):
            """Partial proj + store for one 128-token tile; yields per oc.
            Copies ride the DVE except one per tile on ACT (ACT carries the
            exp cadence; DVE has the headroom)."""
            st = state[b]
            ao = st["ao"]
            for oc in range(4):
                ps_y = p_ps.tile([128, 512], F32, tag="s", bufs=2,
                                 name=f"y{b}{tt}{oc}")
                for ct in range(2):
                    nc.tensor.matmul(ps_y[:], ao[:, ct, tt * 128:(tt + 1) * 128],
                                     wp[:, ct, oc * 512:(oc + 1) * 512],
                                     start=(ct == 0), stop=(ct == 1))
                yt = p_y.tile([128, 512], F16, tag="yt", name=f"yt{b}{tt}{oc}")
                if oc == 1:
                    nc.scalar.copy(yt[:], ps_y[:])
                else:
                    nc.vector.tensor_copy(yt[:], ps_y[:])
                (ring or nc.sync).dma_start(
                    y_h.ap()[b * N + tt * 128:b * N + (tt + 1) * 128,
                             oc * 512:(oc + 1) * 512], yt[:])
                yield

        # ---------------- SDPA ----------------
        sdpa_pend = []   # deferred normalize tails

        def normalize(b, hl, qc, ps_av, rdf):
            st = state[b]
            q0 = qc * 512
            nc.vector.tensor_mul(out=st["ao"][:, hl, q0:q0 + 512],
                                 in0=ps_av[:], in1=rdf[:])

        def sdpa_unit(b, hl, qc, pull):
            """One (batch, head, 512-wide q chunk) SDPA unit.  `pull(n)` emits
            up to n filler work-groups (PE-dense) inside the kt loop."""
            st = state[b]
            if hl == 0 and qc == 0:
                st["ao"] = p_ao.tile([128, 2, N], F16, tag="ao", name=f"ao{b}")
            qT, kT, vtok = st["qT"], st["kT"], st["vtok"]
            q0 = qc * 512
            ex = p_exp.tile([128, 16, 512], F16, tag="ex", name=f"ex{b}{hl}{qc}")
            acc = p_acc.tile([128, 512], F16, tag="acc", name=f"acc{b}{hl}{qc}")
            accp = p_acc.tile([128, 512], F16, tag="accp", name=f"accp{b}{hl}{qc}")
            ps_s = [None] * 16
            ps_av = None

            def qk(kt):
                ps_s[kt] = p_ps.tile([128, 512], F32, tag="s", bufs=2,
                                     name=f"s{b}{hl}{qc}{kt}")
                nc.tensor.matmul(ps_s[kt][:], kT[:, hl, kt * 128:(kt + 1) * 128],
                                 qT[:, hl, q0:q0 + 512], start=True, stop=True)

            def tailpair(j):
                nc.scalar.activation(ex[:, j, :], ps_s[j][:],
                                     mybir.ActivationFunctionType.Exp,
                                     bias=nb4[:], scale=SCALE)
                nc.scalar.activation(ex[:, j + 1, :], ps_s[j + 1][:],
                                     mybir.ActivationFunctionType.Exp,
                                     bias=nb4[:], scale=SCALE)
                for kt in (j, j + 1):
                    nc.tensor.matmul(ps_av[:],
                                     vtok[:, kt, hl * 128:(hl + 1) * 128],
                                     ex[:, kt, :],
                                     start=(kt == 0), stop=(kt == 15))
                # denominator partials: exp-tile sums ride the DVE (kt 0-9)
                # and the Pool engine (kt 10-15), both off the PE
                if j == 0:
                    nc.vector.tensor_add(out=acc[:], in0=ex[:, 0, :],
                                         in1=ex[:, 1, :])
                elif j < 10:
                    nc.vector.tensor_add(out=acc[:], in0=acc[:], in1=ex[:, j, :])
                    nc.vector.tensor_add(out=acc[:], in0=acc[:],
                                         in1=ex[:, j + 1, :])
                elif j == 10:
                    nc.gpsimd.tensor_add(out=accp[:], in0=ex[:, 10, :],
                                         in1=ex[:, 11, :])
                else:
                    nc.gpsimd.tensor_add(out=accp[:], in0=accp[:],
                                         in1=ex[:, j, :])
                    nc.gpsimd.tensor_add(out=accp[:], in0=accp[:],
                                         in1=ex[:, j + 1, :])

            for i in range(8):
                qk(2 * i)
                qk(2 * i + 1)
                if i == 1:
                    if sdpa_pend:
                        normalize(*sdpa_pend.pop())
                    ps_av = p_ps.tile([128, 512], F32, tag="av", bufs=2,
                                      name=f"o{b}{hl}{qc}")
                if i >= 1:
                    tailpair(2 * i - 2)
                pull(1)
            tailpair(14)
            nc.vector.tensor_add(out=acc[:], in0=acc[:], in1=accp[:])
            # denominator: partition-contract, invert, broadcast (Pool)
            ps_d = p_ps.tile([1, 512], F32, tag="row", bufs=2,
                             name=f"d{b}{hl}{qc}")
            nc.tensor.matmul(ps_d[:], onecol[:], acc[:], start=True, stop=True)
            ld = p_row.tile([1, 512], F32, tag="lrow", name=f"ld{b}{hl}{qc}")
            nc.scalar.activation(ld[:], ps_d[:],
                                 mybir.ActivationFunctionType.Ln)
            rd = p_row.tile([1, 512], F32, tag="rd", name=f"rd{b}{hl}{qc}")
            nc.scalar.activation(rd[:], ld[:],
                                 mybir.ActivationFunctionType.Exp, scale=-1.0)
            rdf = p_bcast.tile([128, 512], F32, tag="rdf", name=f"rdf{b}{hl}{qc}")
            nc.gpsimd.partition_broadcast(rdf[:], rd[:])
            sdpa_pend.append((b, hl, qc, ps_av, rdf))

        # ================= emission schedule =================
        # One global FIFO of PE-dense filler generators, pulled from inside
        # the SDPA kt loops: batch-1 QKV chunks first, then proj quarters of
        # each batch as its q-ranges get normalized.
        queue = []

        def pull(n):
            done = 0
            while queue and done < n:
                try:
                    next(queue[0])
                    done += 1
                except StopIteration:
                    queue.pop(0)
            return done

        a_begin(0)
        for ch in range(8):
            for _ in a_chunk_groups(0, ch):
                pass
        a_begin(1)
        queue.extend(a_chunk_groups(1, ch) for ch in range(8))

        # qc-major unit order: a q-range's second head finishes two units
        # after its first, so its proj quarters join the filler queue early
        for b in range(B):
            for qc in range(4):
                for hl in range(HPC):
                    if b == 0:
                        pulls = [1] * 8                      # chunk groups
                    else:
                        pulls = [2, 1] * 4                   # proj quarters
                    it = iter(pulls)
                    sdpa_unit(b, hl, qc, lambda n: pull(n * next(it)))
                if qc >= 1:
                    # proj for the q-range whose both heads are normalized
                    queue.extend(c_quarters(b, tt)
                                 for tt in range(4 * (qc - 1), 4 * qc))
                elif b == 1:
                    # batch-0 qc-3 became available at this phase's first pop
                    queue.extend(c_quarters(0, tt) for tt in range(12, 16))
        normalize(*sdpa_pend.pop())
        pull(10 ** 9)   # drain remaining proj work
        for tt in range(12, 16):
            for _ in c_quarters(1, tt, ring=nc.scalar):
                pass

    nc.compile()
    _CACHE["nc"] = nc
    return nc


def make_in_maps(x, rope, qkv_w, qkv_b, proj_w, q_norm_w, k_norm_w):
    """Host-side prep: transpose x, slice/scale weights per core."""
    x = np.asarray(x, np.float32)
    rope = np.asarray(rope, np.float32)
    qkv_w = np.asarray(qkv_w, np.float32)
    qkv_b = np.asarray(qkv_b, np.float32)
    proj_w = np.asarray(proj_w, np.float32)
    g_q = np.asarray(q_norm_w, np.float32)
    g_k = np.asarray(k_norm_w, np.float32)
    if np.any(g_q == 0) or np.any(g_k == 0):
        raise ValueError("zero rmsnorm weight not supported")

    xt = np.ascontiguousarray(x.reshape(TOK, C).T.astype(np.float16))  # [C, TOK]
    cos = np.cos(rope)                                        # [N, 64]
    sin = np.sin(rope)
    cos2 = np.ascontiguousarray(
        np.concatenate([cos, cos], axis=1).T.astype(np.float16))       # [128, N]
    sin2 = np.ascontiguousarray(
        np.concatenate([-sin, sin], axis=1).T.astype(np.float16))      # [128, N]
    invg2 = np.stack([1.0 / g_q ** 2, 1.0 / g_k ** 2], axis=1).astype(np.float16)
    onecol = np.ones((128, 1), np.float16)
    eps = np.full((1, 1), EPS, np.float32)
    nb4 = np.full((128, 1), ESHIFT, np.float32)

    in_maps = []
    for c in range(NCORES):
        hs = [HPC * c + hl for hl in range(HPC)]
        # chan-tiles: q_h0, q_h1, k_h0, k_h1 (g-scaled rows + bias)
        rows, biases = [], []
        for base, g in ((0, g_q), (C, g_k)):
            for h in hs:
                r0 = base + h * D
                rows.append(qkv_w[r0:r0 + D] * g[:, None])
                biases.append(qkv_b[r0:r0 + D] * g)
        wqk = np.ascontiguousarray(
            np.concatenate(rows, axis=0).T.astype(np.float16))           # [C, 512]
        qkb = np.stack(biases, axis=1)                                   # [128, 4]
        vrows = [qkv_w[2 * C + h * D:2 * C + (h + 1) * D] for h in hs]
        wv = np.ascontiguousarray(
            np.concatenate(vrows, axis=0).T.astype(np.float16))          # [C, 256]
        vbias = np.concatenate(
            [qkv_b[2 * C + h * D:2 * C + (h + 1) * D] for h in hs])      # [256]
        vb = np.broadcast_to(vbias, (128, 256)).astype(np.float32).copy()
        cols = np.concatenate([np.arange(h * D, (h + 1) * D) for h in hs])
        wpT = np.ascontiguousarray(proj_w[:, cols].T.astype(np.float16))  # [256, C]
        in_maps.append({
            "xt": xt, "wqk": wqk, "wv": wv, "wp": wpT,
            "cos2": cos2, "sin2": sin2, "qkb": qkb, "vb": vb,
            "invg2": invg2, "onecol": onecol, "eps": eps, "nb4": nb4,
        })
    return in_maps


def kernel(x, rope, qkv_w, qkv_b, proj_w, proj_b, q_norm_w, k_norm_w):
    nc = build_module()
    in_maps = make_in_maps(x, rope, qkv_w, qkv_b, proj_w, q_norm_w, k_norm_w)
    res = bass_utils.run_bass_kernel_spmd(nc, in_maps,
                                          core_ids=list(range(NCORES)), **RUN_KW)
    _CACHE["last_result"] = res
    y = np.zeros((TOK, C), np.float64)
    for c in range(NCORES):
        y += res.results[c]["y"].astype(np.float64)
    y += np.asarray(proj_b, np.float32).astype(np.float64)
    return y.astype(np.float32).reshape(B, N, C)


# revision 19
# speedup vs baseline: 1.2525x; 1.2525x over previous
"""Multi-head attention (16 heads, D=128) on 8 trn2 NeuronCores.

Sharding: tensor-parallel over heads — each core owns 2 heads.
Per core: qkv projection for its 768 channels (chan-major for q/k,
token-major for v), fused RMSNorm+RoPE on q/k, SDPA in transposed-score
layout, partial proj over its 256 channels.  Host sums the 8 partial
outputs + bias.

Matmul operands are fp16; accumulation fp32 in PSUM.
exp is computed as exp(s/sqrt(D) - 4) — softmax-invariant shift that
keeps fp16 exp values in range.

Softmax denominator: exp tiles are summed across the 16 k-tiles on the
DVE (serial in-place accumulate), contracted over partitions by a single
[128,1]-ones matmul, inverted via Ln/Exp on [1,512], and broadcast to
all 128 partitions on the (otherwise idle) Pool engine — this keeps the
PE free of the 16-per-chunk ones-matmuls the previous version used.
The RMSNorm 1/rms row broadcast uses the same Pool path.

Cross-stage software pipeline: the per-engine queues execute in order,
so PE-heavy filler work (next batch's QKV chunks, previous batch's proj
tiles) is interleaved INSIDE each SDPA q-chunk's kt loop.  This keeps
the PE issue stream dense while the ACT engine works through the exp
cadence, instead of the PE head-blocking on a score bank whose exp
hasn't drained.
"""
import math
from contextlib import ExitStack

import numpy as np

import concourse.bass as bass
import concourse.mybir as mybir
import concourse.tile as tile
from concourse import bacc, bass_utils

F32 = mybir.dt.float32
F16 = mybir.dt.float16

H, D, B, N, C = 16, 128, 2, 2048, 2048
NCORES = 8
HPC = H // NCORES            # heads per core = 2
TOK = B * N                  # 4096
EPS = float(np.finfo(np.float32).eps)
SCALE = 1.0 / math.sqrt(D)
ESHIFT = -4.0                # exp(s*SCALE + ESHIFT); softmax-invariant

_CACHE = {}
RUN_KW = {}   # test.py sets {"trace": True}


def _pin_act_table():
    """Restrict Exp/Ln to the combined natural_log_exp_and_others set so the
    table-load pass keeps a single ACT table resident."""
    import concourse.hw_specs as hw
    tabs = hw.get_activation_tables("gen3")
    for name, funcs in tabs.items():
        if name != "natural_log_exp_and_others":
            funcs.discard(mybir.ActivationFunctionType.Exp)
            funcs.discard(mybir.ActivationFunctionType.Ln)


def build_module():
    """Build + compile the per-core Bass module (same NEFF for all cores)."""
    if "nc" in _CACHE:
        return _CACHE["nc"]
    _pin_act_table()
    nc = bacc.Bacc("TRN2", target_bir_lowering=False, debug=False,
                   num_devices=NCORES)

    xt_h = nc.dram_tensor("xt", [C, TOK], F16, kind="ExternalInput")
    wqk_h = nc.dram_tensor("wqk", [C, 4 * 128], F16, kind="ExternalInput")
    wv_h = nc.dram_tensor("wv", [C, 2 * 128], F16, kind="ExternalInput")
    wp_h = nc.dram_tensor("wp", [2 * 128, C], F16, kind="ExternalInput")
    cos2_h = nc.dram_tensor("cos2", [128, N], F16, kind="ExternalInput")
    sin2_h = nc.dram_tensor("sin2", [128, N], F16, kind="ExternalInput")
    qkb_h = nc.dram_tensor("qkb", [128, 4], F32, kind="ExternalInput")
    vb_h = nc.dram_tensor("vb", [128, 256], F32, kind="ExternalInput")
    invg2_h = nc.dram_tensor("invg2", [128, 2], F16, kind="ExternalInput")
    onecol_h = nc.dram_tensor("onecol", [128, 1], F16, kind="ExternalInput")
    eps_h = nc.dram_tensor("eps", [1, 1], F32, kind="ExternalInput")
    nb4_h = nc.dram_tensor("nb4", [128, 1], F32, kind="ExternalInput")
    y_h = nc.dram_tensor("y", [TOK, C], F16, kind="ExternalOutput")

    with tile.TileContext(nc) as tc, ExitStack() as ctx:
        pc = ctx.enter_context(tc.tile_pool(name="consts", bufs=1))
        p_xt = ctx.enter_context(tc.tile_pool(name="xt", bufs=4))
        p_qkv = ctx.enter_context(tc.tile_pool(name="qkv", bufs=2))
        p_qraw = ctx.enter_context(tc.tile_pool(name="qraw", bufs=2))
        p_qsw = ctx.enter_context(tc.tile_pool(name="qsw", bufs=3))
        p_sq = ctx.enter_context(tc.tile_pool(name="sq", bufs=2))
        p_exp = ctx.enter_context(tc.tile_pool(name="exp", bufs=2))
        p_acc = ctx.enter_context(tc.tile_pool(name="acc", bufs=2))
        p_bcast = ctx.enter_context(tc.tile_pool(name="bcast", bufs=2))
        p_ao = ctx.enter_context(tc.tile_pool(name="ao", bufs=2))
        p_y = ctx.enter_context(tc.tile_pool(name="y", bufs=6))
        p_row = ctx.enter_context(tc.tile_pool(name="rows", bufs=2))
        # PSUM: qk(2) + s(2) + av(2) + row(2) = 8 banks
        p_ps = ctx.enter_context(tc.tile_pool(name="ps", bufs=8, space="PSUM"))

        # ---- constants / weights ----
        # Startup critical path: the sync ring carries only wqk half-0 and the
        # first xt tiles (what the first matmuls wait on); everything else
        # rides the ACT engine's DMA ring, which is idle at startup.
        wqk = pc.tile([128, 16, 512], F16)
        wv = pc.tile([128, 16, 256], F16)
        nc.sync.dma_start(wqk[:, 0:8, :],
                          wqk_h.ap()[0:1024].rearrange("(t p) j -> p t j", p=128))
        nc.scalar.dma_start(wqk[:, 8:16, :],
                            wqk_h.ap()[1024:2048].rearrange("(t p) j -> p t j", p=128))
        qkb = pc.tile([128, 4], F32)
        nc.scalar.dma_start(qkb[:], qkb_h.ap())
        vb = pc.tile([128, 256], F32)
        nc.scalar.dma_start(vb[:], vb_h.ap())
        invg2 = pc.tile([128, 2], F16)
        nc.scalar.dma_start(invg2[:], invg2_h.ap())
        eps_t = pc.tile([1, 1], F32)
        nc.scalar.dma_start(eps_t[:], eps_h.ap())
        onecol = pc.tile([128, 1], F16)
        nc.scalar.dma_start(onecol[:], onecol_h.ap())
        nb4 = pc.tile([128, 1], F32)
        nc.scalar.dma_start(nb4[:], nb4_h.ap())

        cos2 = pc.tile([128, N], F16)
        sin2 = pc.tile([128, N], F16)
        wp = pc.tile([128, 2, 2048], F16)

        def load_late_consts():
            # emitted after the first chunk's xt DMAs; ACT ring, off the
            # startup critical path
            for hf in range(2):
                nc.scalar.dma_start(wv[:, hf * 8:(hf + 1) * 8, :],
                                    wv_h.ap()[hf * 1024:(hf + 1) * 1024]
                                    .rearrange("(t p) j -> p t j", p=128))
            nc.scalar.dma_start(cos2[:], cos2_h.ap())
            nc.scalar.dma_start(sin2[:], sin2_h.ap())
            nc.scalar.dma_start(wp[:], wp_h.ap().rearrange("(t p) j -> p t j", p=128))

        # ---------------- per-batch state ----------------
        state = {}

        def ph1(b, qraw_g, g0, gi):
            """sumsq + 1/rms rows for one 512-token group (4 chan-tiles)."""
            rrows = []
            for ct in range(4):
                is_k = ct // 2
                src_q = qraw_g[:, ct, :]
                sq = p_sq.tile([128, 512], F16, tag="sq", name=f"sq{b}{gi}{ct}")
                nc.vector.tensor_mul(out=sq[:], in0=src_q, in1=src_q)
                ps_ss = p_ps.tile([1, 512], F32, tag="row", bufs=2,
                                  name=f"ss{b}{gi}{ct}")
                nc.tensor.matmul(ps_ss[:], invg2[:, is_k:is_k + 1], sq[:],
                                 start=True, stop=True)
                # rrow = 1/sqrt(var+eps) = exp(-0.5*ln(var+eps))
                lrow = p_row.tile([1, 512], F32, tag="lrow", name=f"lr{b}{gi}{ct}")
                nc.scalar.activation(lrow[:], ps_ss[:],
                                     mybir.ActivationFunctionType.Ln,
                                     bias=eps_t[:], scale=1.0 / D)
                rrow = p_row.tile([1, 512], F16, tag="recip", bufs=4,
                                  name=f"rr{b}{gi}{ct}")
                nc.scalar.activation(rrow[:], lrow[:],
                                     mybir.ActivationFunctionType.Exp,
                                     scale=-0.5)
                rrows.append(rrow)
            return rrows

        def ph2(b, qraw_g, g0, gi, rrows):
            """RoPE + rms-scale for one 512-token group."""
            st = state[b]
            for ct in range(4):
                hl, is_k = ct % 2, ct // 2
                dst = (st["kT"] if is_k else st["qT"])
                src_q = qraw_g[:, ct, :]
                rsf = p_bcast.tile([128, 512], F16, tag="rsf",
                                   name=f"rsf{b}{gi}{ct}")
                nc.gpsimd.partition_broadcast(rsf[:], rrows[ct][:])
                qsw = p_qsw.tile([128, 512], F16, tag="qsw", name=f"qsw{b}{gi}{ct}")
                nc.sync.dma_start(qsw[0:64, :], src_q[64:128, :])
                nc.sync.dma_start(qsw[64:128, :], src_q[0:64, :])
                # in-place: qc into qraw, qs into qsw
                nc.vector.tensor_mul(out=src_q, in0=src_q,
                                     in1=cos2[:, g0:g0 + 512])
                nc.vector.tensor_mul(out=qsw[:], in0=qsw[:],
                                     in1=sin2[:, g0:g0 + 512])
                rot = dst[:, hl, g0:g0 + 512]
                nc.vector.tensor_add(out=rot, in0=src_q, in1=qsw[:])
                nc.vector.tensor_mul(out=rot, in0=rot, in1=rsf[:])

        def a_begin(b):
            st = state[b] = {}
            st["qT"] = p_qkv.tile([128, HPC, N], F16, tag="qT", name=f"qT{b}")
            st["kT"] = p_qkv.tile([128, HPC, N], F16, tag="kT", name=f"kT{b}")
            st["vtok"] = p_qkv.tile([128, 16, 256], F16, tag="v", name=f"v{b}")
            st["qraw"] = None
            st["ph1_pend"] = []
            st["ph2_pend"] = []

        def a_chunk_groups(b, ch):
            """QKV production for one 256-token chunk, as a generator that
            yields after each ~1.7us matmul group (6 groups per chunk)."""
            st = state[b]
            tok0 = b * N + ch * 256
            if ch % 2 == 0:
                st["qraw"] = p_qraw.tile([128, 4, 512], F16, tag="qraw",
                                         name=f"qraw{b}{ch}")
            qraw = st["qraw"]
            off = (ch % 2) * 256
            xts = []
            for half in range(2):
                xt = p_xt.tile([128, 8, 256], F16, tag="xt",
                               name=f"xt{b}{ch}{half}")
                src = xt_h.ap()[half * 1024:(half + 1) * 1024, tok0:tok0 + 256]
                nc.sync.dma_start(xt[:], src.rearrange("(t p) j -> p t j", p=128))
                xts.append(xt)
            if b == 0 and ch == 0:
                load_late_consts()
            # q/k chan-tiles, one PSUM bank each, sequential over ct
            for ct in range(4):
                ps_qk = p_ps.tile([128, 256], F32, tag="qk", bufs=2,
                                  name=f"a{b}{ch}{ct}")
                for half in range(2):
                    for kt in range(8):
                        nc.tensor.matmul(
                            ps_qk[:], wqk[:, half * 8 + kt, ct * 128:(ct + 1) * 128],
                            xts[half][:, kt, :],
                            start=(half == 0 and kt == 0),
                            stop=(half == 1 and kt == 7))
                nc.vector.tensor_scalar_add(qraw[:, ct, off:off + 256],
                                            ps_qk[:], qkb[:, ct:ct + 1])
                # inject deferred norm work so its ACT/PE latency hides
                if ct == 1 and st["ph1_pend"]:
                    args = st["ph1_pend"].pop()
                    st["ph2_pend"].append((args[0], args[1], args[2],
                                           ph1(b, *args)))
                elif ct == 3 and st["ph2_pend"]:
                    ph2(b, *st["ph2_pend"].pop())
                yield
            for s in range(2):
                ps_v = p_ps.tile([128, 256], F32, tag="qk", bufs=2,
                                 name=f"av{b}{ch}{s}")
                for half in range(2):
                    for kt in range(8):
                        nc.tensor.matmul(
                            ps_v[:], xts[half][:, kt, s * 128:(s + 1) * 128],
                            wv[:, half * 8 + kt, :],
                            start=(half == 0 and kt == 0),
                            stop=(half == 1 and kt == 7))
                nc.vector.tensor_add(out=st["vtok"][:, ch * 2 + s, :],
                                     in0=ps_v[:], in1=vb[:])
                yield
            if ch % 2 == 1:
                st["ph1_pend"].append((qraw, (ch - 1) * 256, ch // 2))
            if ch == 7:
                # flush the last group's norm epilogue
                args = st["ph1_pend"].pop()
                ph2(b, args[0], args[1], args[2], ph1(b, *args))

        # ---------------- proj ----------------
        def c_quarters(b, tt, ring=None):
            """Partial proj + store for one 128-token tile; yields per oc.
            Copies ride the DVE except one per tile on ACT (ACT carries the
            exp cadence; DVE has the headroom)."""
            st = state[b]
            ao = st["ao"]
            for oc in range(4):
                ps_y = p_ps.tile([128, 512], F32, tag="s", bufs=2,
                                 name=f"y{b}{tt}{oc}")
                for ct in range(2):
                    nc.tensor.matmul(ps_y[:], ao[:, ct, tt * 128:(tt + 1) * 128],
                                     wp[:, ct, oc * 512:(oc + 1) * 512],
                                     start=(ct == 0), stop=(ct == 1))
                yt = p_y.tile([128, 512], F16, tag="yt", name=f"yt{b}{tt}{oc}")
                if oc == 1 or (oc == 3 and tt % 2 == 1):
                    nc.scalar.copy(yt[:], ps_y[:])
                else:
                    nc.vector.tensor_copy(yt[:], ps_y[:])
                (ring or nc.sync).dma_start(
                    y_h.ap()[b * N + tt * 128:b * N + (tt + 1) * 128,
                             oc * 512:(oc + 1) * 512], yt[:])
                yield

        # ---------------- SDPA ----------------
        sdpa_pend = []   # deferred normalize tails

        def normalize(b, hl, qc, ps_av, rdf):
            st = state[b]
            q0 = qc * 512
            nc.vector.tensor_mul(out=st["ao"][:, hl, q0:q0 + 512],
                                 in0=ps_av[:], in1=rdf[:])

        def sdpa_unit(b, hl, qc, pull):
            """One (batch, head, 512-wide q chunk) SDPA unit.  `pull(n)` emits
            up to n filler work-groups (PE-dense) inside the kt loop."""
            st = state[b]
            if hl == 0 and qc == 0:
                st["ao"] = p_ao.tile([128, 2, N], F16, tag="ao", name=f"ao{b}")
            qT, kT, vtok = st["qT"], st["kT"], st["vtok"]
            q0 = qc * 512
            ex = p_exp.tile([128, 16, 512], F16, tag="ex", name=f"ex{b}{hl}{qc}")
            acc = p_acc.tile([128, 512], F16, tag="acc", name=f"acc{b}{hl}{qc}")
            ps_s = [None] * 16
            ps_av = None

            def qk(kt):
                ps_s[kt] = p_ps.tile([128, 512], F32, tag="s", bufs=2,
                                     name=f"s{b}{hl}{qc}{kt}")
                nc.tensor.matmul(ps_s[kt][:], kT[:, hl, kt * 128:(kt + 1) * 128],
                                 qT[:, hl, q0:q0 + 512], start=True, stop=True)

            def tailpair(j):
                nc.scalar.activation(ex[:, j, :], ps_s[j][:],
                                     mybir.ActivationFunctionType.Exp,
                                     bias=nb4[:], scale=SCALE)
                nc.scalar.activation(ex[:, j + 1, :], ps_s[j + 1][:],
                                     mybir.ActivationFunctionType.Exp,
                                     bias=nb4[:], scale=SCALE)
                for kt in (j, j + 1):
                    nc.tensor.matmul(ps_av[:],
                                     vtok[:, kt, hl * 128:(hl + 1) * 128],
                                     ex[:, kt, :],
                                     start=(kt == 0), stop=(kt == 15))
                # denominator partial: acc += exp tile (DVE, off the PE; the
                # Pool engine must stay broadcast-only — mixing op types
                # thrashes its ucode library, ~1us per switch)
                if j == 0:
                    nc.vector.tensor_add(out=acc[:], in0=ex[:, 0, :],
                                         in1=ex[:, 1, :])
                else:
                    nc.vector.tensor_add(out=acc[:], in0=acc[:], in1=ex[:, j, :])
                    nc.vector.tensor_add(out=acc[:], in0=acc[:],
                                         in1=ex[:, j + 1, :])

            for i in range(8):
                qk(2 * i)
                qk(2 * i + 1)
                if i == 1:
                    if sdpa_pend:
                        normalize(*sdpa_pend.pop())
                    ps_av = p_ps.tile([128, 512], F32, tag="av", bufs=2,
                                      name=f"o{b}{hl}{qc}")
                if i >= 1:
                    tailpair(2 * i - 2)
                pull(1)
            tailpair(14)
            # denominator: partition-contract, invert, broadcast (Pool)
            ps_d = p_ps.tile([1, 512], F32, tag="row", bufs=2,
                             name=f"d{b}{hl}{qc}")
            nc.tensor.matmul(ps_d[:], onecol[:], acc[:], start=True, stop=True)
            ld = p_row.tile([1, 512], F32, tag="lrow", name=f"ld{b}{hl}{qc}")
            nc.scalar.activation(ld[:], ps_d[:],
                                 mybir.ActivationFunctionType.Ln)
            rd = p_row.tile([1, 512], F32, tag="rd", name=f"rd{b}{hl}{qc}")
            nc.scalar.activation(rd[:], ld[:],
                                 mybir.ActivationFunctionType.Exp, scale=-1.0)
            rdf = p_bcast.tile([128, 512], F32, tag="rdf", name=f"rdf{b}{hl}{qc}")
            nc.gpsimd.partition_broadcast(rdf[:], rd[:])
            sdpa_pend.append((b, hl, qc, ps_av, rdf))

        # ================= emission schedule =================
        # One global FIFO of PE-dense filler generators, pulled from inside
        # the SDPA kt loops: batch-1 QKV chunks first, then proj quarters of
        # each batch as its q-ranges get normalized.
        queue = []

        def pull(n):
            done = 0
            while queue and done < n:
                try:
                    next(queue[0])
                    done += 1
                except StopIteration:
                    queue.pop(0)
            return done

        a_begin(0)
        for ch in range(8):
            for _ in a_chunk_groups(0, ch):
                pass
        a_begin(1)
        queue.extend(a_chunk_groups(1, ch) for ch in range(8))

        # qc-major unit order: a q-range's second head finishes two units
        # after its first, so its proj quarters join the filler queue early
        for b in range(B):
            for qc in range(4):
                for hl in range(HPC):
                    if b == 0:
                        pulls = [1] * 8                      # chunk groups
                    else:
                        pulls = [2, 1] * 4                   # proj quarters
                    it = iter(pulls)
                    sdpa_unit(b, hl, qc, lambda n: pull(n * next(it)))
                if qc >= 1:
                    # proj for the q-range whose both heads are normalized
                    queue.extend(c_quarters(b, tt)
                                 for tt in range(4 * (qc - 1), 4 * qc))
                elif b == 1:
                    # batch-0 qc-3 became available at this phase's first pop
                    queue.extend(c_quarters(0, tt) for tt in range(12, 16))
        normalize(*sdpa_pend.pop())
        pull(10 ** 9)   # drain remaining proj work
        for tt in range(12, 16):
            for _ in c_quarters(1, tt, ring=nc.scalar):
                pass

    nc.compile()
    _CACHE["nc"] = nc
    return nc


def make_in_maps(x, rope, qkv_w, qkv_b, proj_w, q_norm_w, k_norm_w):
    """Host-side prep: transpose x, slice/scale weights per core."""
    x = np.asarray(x, np.float32)
    rope = np.asarray(rope, np.float32)
    qkv_w = np.asarray(qkv_w, np.float32)
    qkv_b = np.asarray(qkv_b, np.float32)
    proj_w = np.asarray(proj_w, np.float32)
    g_q = np.asarray(q_norm_w, np.float32)
    g_k = np.asarray(k_norm_w, np.float32)
    if np.any(g_q == 0) or np.any(g_k == 0):
        raise ValueError("zero rmsnorm weight not supported")

    xt = np.ascontiguousarray(x.reshape(TOK, C).T.astype(np.float16))  # [C, TOK]
    cos = np.cos(rope)                                        # [N, 64]
    sin = np.sin(rope)
    cos2 = np.ascontiguousarray(
        np.concatenate([cos, cos], axis=1).T.astype(np.float16))       # [128, N]
    sin2 = np.ascontiguousarray(
        np.concatenate([-sin, sin], axis=1).T.astype(np.float16))      # [128, N]
    invg2 = np.stack([1.0 / g_q ** 2, 1.0 / g_k ** 2], axis=1).astype(np.float16)
    onecol = np.ones((128, 1), np.float16)
    eps = np.full((1, 1), EPS, np.float32)
    nb4 = np.full((128, 1), ESHIFT, np.float32)

    in_maps = []
    for c in range(NCORES):
        hs = [HPC * c + hl for hl in range(HPC)]
        # chan-tiles: q_h0, q_h1, k_h0, k_h1 (g-scaled rows + bias)
        rows, biases = [], []
        for base, g in ((0, g_q), (C, g_k)):
            for h in hs:
                r0 = base + h * D
                rows.append(qkv_w[r0:r0 + D] * g[:, None])
                biases.append(qkv_b[r0:r0 + D] * g)
        wqk = np.ascontiguousarray(
            np.concatenate(rows, axis=0).T.astype(np.float16))           # [C, 512]
        qkb = np.stack(biases, axis=1)                                   # [128, 4]
        vrows = [qkv_w[2 * C + h * D:2 * C + (h + 1) * D] for h in hs]
        wv = np.ascontiguousarray(
            np.concatenate(vrows, axis=0).T.astype(np.float16))          # [C, 256]
        vbias = np.concatenate(
            [qkv_b[2 * C + h * D:2 * C + (h + 1) * D] for h in hs])      # [256]
        vb = np.broadcast_to(vbias, (128, 256)).astype(np.float32).copy()
        cols = np.concatenate([np.arange(h * D, (h + 1) * D) for h in hs])
        wpT = np.ascontiguousarray(proj_w[:, cols].T.astype(np.float16))  # [256, C]
        in_maps.append({
            "xt": xt, "wqk": wqk, "wv": wv, "wp": wpT,
            "cos2": cos2, "sin2": sin2, "qkb": qkb, "vb": vb,
            "invg2": invg2, "onecol": onecol, "eps": eps, "nb4": nb4,
        })
    return in_maps


def kernel(x, rope, qkv_w, qkv_b, proj_w, proj_b, q_norm_w, k_norm_w):
    nc = build_module()
    in_maps = make_in_maps(x, rope, qkv_w, qkv_b, proj_w, q_norm_w, k_norm_w)
    res = bass_utils.run_bass_kernel_spmd(nc, in_maps,
                                          core_ids=list(range(NCORES)), **RUN_KW)
    _CACHE["last_result"] = res
    y = np.zeros((TOK, C), np.float64)
    for c in range(NCORES):
        y += res.results[c]["y"].astype(np.float64)
    y += np.asarray(proj_b, np.float32).astype(np.float64)
    return y.astype(np.float32).reshape(B, N, C)


# revision 25
# speedup vs baseline: 1.3205x; 1.0543x over previous
"""Multi-head attention (16 heads, D=128) on 8 trn2 NeuronCores.

Sharding: tensor-parallel over heads — each core owns 2 heads.
Per core: qkv projection for its 768 channels (chan-major for q/k,
token-major for v), fused RMSNorm+RoPE on q/k, SDPA in transposed-score
layout, partial proj over its 256 channels.  Host sums the 8 partial
outputs + bias.

Matmul operands are fp16; accumulation fp32 in PSUM.
exp is computed as exp(s/sqrt(D) - 4) — softmax-invariant shift that
keeps fp16 exp values in range.

Softmax denominator: exp tiles are summed across the 16 k-tiles on the
DVE (serial in-place accumulate), contracted over partitions by a single
[128,1]-ones matmul, inverted via Ln/Exp on [1,512], and broadcast to
all 128 partitions on the (otherwise idle) Pool engine — this keeps the
PE free of the 16-per-chunk ones-matmuls the previous version used.
The RMSNorm 1/rms row broadcast uses the same Pool path.

Cross-stage software pipeline: the per-engine queues execute in order,
so PE-heavy filler work (next batch's QKV chunks, previous batch's proj
tiles) is interleaved INSIDE each SDPA q-chunk's kt loop.  This keeps
the PE issue stream dense while the ACT engine works through the exp
cadence, instead of the PE head-blocking on a score bank whose exp
hasn't drained.
"""
import math
from contextlib import ExitStack

import numpy as np

import concourse.bass as bass
import concourse.mybir as mybir
import concourse.tile as tile
from concourse import bacc, bass_utils

F32 = mybir.dt.float32
F16 = mybir.dt.float16

H, D, B, N, C = 16, 128, 2, 2048, 2048
NCORES = 8
HPC = H // NCORES            # heads per core = 2
TOK = B * N                  # 4096
EPS = float(np.finfo(np.float32).eps)
SCALE = 1.0 / math.sqrt(D)
ESHIFT = -4.0                # exp(s*SCALE + ESHIFT); softmax-invariant

_CACHE = {}
RUN_KW = {}   # test.py sets {"trace": True}


def _pin_act_table():
    """Restrict Exp/Ln to the combined natural_log_exp_and_others set so the
    table-load pass keeps a single ACT table resident."""
    import concourse.hw_specs as hw
    tabs = hw.get_activation_tables("gen3")
    for name, funcs in tabs.items():
        if name != "natural_log_exp_and_others":
            funcs.discard(mybir.ActivationFunctionType.Exp)
            funcs.discard(mybir.ActivationFunctionType.Ln)


def build_module():
    """Build + compile the per-core Bass module (same NEFF for all cores)."""
    if "nc" in _CACHE:
        return _CACHE["nc"]
    _pin_act_table()
    nc = bacc.Bacc("TRN2", target_bir_lowering=False, debug=False,
                   num_devices=NCORES)

    xt_h = nc.dram_tensor("xt", [C, TOK], F16, kind="ExternalInput")
    wqk_h = nc.dram_tensor("wqk", [C, 4 * 128], F16, kind="ExternalInput")
    wv_h = nc.dram_tensor("wv", [C, 2 * 128], F16, kind="ExternalInput")
    wp_h = nc.dram_tensor("wp", [2 * 128, C], F16, kind="ExternalInput")
    cos2_h = nc.dram_tensor("cos2", [128, N], F16, kind="ExternalInput")
    sin2_h = nc.dram_tensor("sin2", [128, N], F16, kind="ExternalInput")
    qkb_h = nc.dram_tensor("qkb", [128, 4], F32, kind="ExternalInput")
    vb_h = nc.dram_tensor("vb", [128, 256], F32, kind="ExternalInput")
    invg2_h = nc.dram_tensor("invg2", [128, 2], F16, kind="ExternalInput")
    onecol_h = nc.dram_tensor("onecol", [128, 1], F16, kind="ExternalInput")
    eps_h = nc.dram_tensor("eps", [1, 1], F32, kind="ExternalInput")
    nb4_h = nc.dram_tensor("nb4", [128, 1], F32, kind="ExternalInput")
    y_h = nc.dram_tensor("y", [TOK, C], F16, kind="ExternalOutput")

    with tile.TileContext(nc) as tc, ExitStack() as ctx:
        pc = ctx.enter_context(tc.tile_pool(name="consts", bufs=1))
        p_xt = ctx.enter_context(tc.tile_pool(name="xt", bufs=4))
        p_qkv = ctx.enter_context(tc.tile_pool(name="qkv", bufs=2))
        p_qraw = ctx.enter_context(tc.tile_pool(name="qraw", bufs=2))
        p_qsw = ctx.enter_context(tc.tile_pool(name="qsw", bufs=3))
        p_sq = ctx.enter_context(tc.tile_pool(name="sq", bufs=2))
        p_exp = ctx.enter_context(tc.tile_pool(name="exp", bufs=2))
        p_acc = ctx.enter_context(tc.tile_pool(name="acc", bufs=2))
        p_bcast = ctx.enter_context(tc.tile_pool(name="bcast", bufs=2))
        p_ao = ctx.enter_context(tc.tile_pool(name="ao", bufs=2))
        p_y = ctx.enter_context(tc.tile_pool(name="y", bufs=6))
        p_row = ctx.enter_context(tc.tile_pool(name="rows", bufs=2))
        # PSUM: sw(2x2) + av(2) + qk(2, shared with proj/row tiles) = 8 banks
        p_ps = ctx.enter_context(tc.tile_pool(name="ps", bufs=8, space="PSUM"))

        # ---- constants / weights ----
        # Startup critical path: the sync ring carries only wqk half-0 and the
        # first xt tiles (what the first matmuls wait on); everything else
        # rides the ACT engine's DMA ring, which is idle at startup.
        wqk = pc.tile([128, 16, 512], F16)
        wv = pc.tile([128, 16, 256], F16)
        # only the first chan-tile's weights gate the first matmul group; the
        # rest of half-0 follows the first xt tiles (HBM is startup-saturated
        # by all 8 cores, so less-before-first-matmul = earlier first-matmul)
        nc.sync.dma_start(wqk[:, 0:8, 0:128],
                          wqk_h.ap()[0:1024, 0:128].rearrange("(t p) j -> p t j", p=128))
        nc.scalar.dma_start(wqk[:, 8:16, :],
                            wqk_h.ap()[1024:2048].rearrange("(t p) j -> p t j", p=128))
        qkb = pc.tile([128, 4], F32)
        nc.scalar.dma_start(qkb[:], qkb_h.ap())
        vb = pc.tile([128, 256], F32)
        nc.scalar.dma_start(vb[:], vb_h.ap())
        invg2 = pc.tile([128, 2], F16)
        nc.scalar.dma_start(invg2[:], invg2_h.ap())
        eps_t = pc.tile([1, 1], F32)
        nc.scalar.dma_start(eps_t[:], eps_h.ap())
        onecol = pc.tile([128, 1], F16)
        nc.scalar.dma_start(onecol[:], onecol_h.ap())
        nb4 = pc.tile([128, 1], F32)
        nc.scalar.dma_start(nb4[:], nb4_h.ap())

        cos2 = pc.tile([128, N], F16)
        sin2 = pc.tile([128, N], F16)
        wp = pc.tile([128, 2, 2048], F16)

        def load_late_consts():
            # emitted after the first chunk's xt DMAs; ACT ring, off the
            # startup critical path
            for hf in range(2):
                nc.scalar.dma_start(wv[:, hf * 8:(hf + 1) * 8, :],
                                    wv_h.ap()[hf * 1024:(hf + 1) * 1024]
                                    .rearrange("(t p) j -> p t j", p=128))
            nc.scalar.dma_start(cos2[:], cos2_h.ap())
            nc.scalar.dma_start(sin2[:], sin2_h.ap())
            nc.scalar.dma_start(wp[:], wp_h.ap().rearrange("(t p) j -> p t j", p=128))

        # ---------------- per-batch state ----------------
        state = {}

        def ph1(b, qraw_g, g0, gi):
            """sumsq + 1/rms rows for one 512-token group (4 chan-tiles)."""
            rrows = []
            for ct in range(4):
                is_k = ct // 2
                src_q = qraw_g[:, ct, :]
                sq = p_sq.tile([128, 512], F16, tag="sq", name=f"sq{b}{gi}{ct}")
                nc.vector.tensor_mul(out=sq[:], in0=src_q, in1=src_q)
                ps_ss = p_ps.tile([1, 512], F32, tag="qk", bufs=2,
                                  name=f"ss{b}{gi}{ct}")
                nc.tensor.matmul(ps_ss[:], invg2[:, is_k:is_k + 1], sq[:],
                                 start=True, stop=True)
                # rrow = 1/sqrt(var+eps) = exp(-0.5*ln(var+eps))
                lrow = p_row.tile([1, 512], F32, tag="lrow", name=f"lr{b}{gi}{ct}")
                nc.scalar.activation(lrow[:], ps_ss[:],
                                     mybir.ActivationFunctionType.Ln,
                                     bias=eps_t[:], scale=1.0 / D)
                rrow = p_row.tile([1, 512], F16, tag="recip", bufs=4,
                                  name=f"rr{b}{gi}{ct}")
                nc.scalar.activation(rrow[:], lrow[:],
                                     mybir.ActivationFunctionType.Exp,
                                     scale=-0.5)
                rrows.append(rrow)
            return rrows

        def ph2(b, qraw_g, g0, gi, rrows):
            """RoPE + rms-scale for one 512-token group."""
            st = state[b]
            for ct in range(4):
                hl, is_k = ct % 2, ct // 2
                dst = (st["kT"] if is_k else st["qT"])
                src_q = qraw_g[:, ct, :]
                rsf = p_bcast.tile([128, 512], F16, tag="rsf",
                                   name=f"rsf{b}{gi}{ct}")
                nc.gpsimd.partition_broadcast(rsf[:], rrows[ct][:])
                qsw = p_qsw.tile([128, 512], F16, tag="qsw", name=f"qsw{b}{gi}{ct}")
                nc.sync.dma_start(qsw[0:64, :], src_q[64:128, :])
                nc.sync.dma_start(qsw[64:128, :], src_q[0:64, :])
                # in-place: qc into qraw, qs into qsw
                nc.vector.tensor_mul(out=src_q, in0=src_q,
                                     in1=cos2[:, g0:g0 + 512])
                nc.vector.tensor_mul(out=qsw[:], in0=qsw[:],
                                     in1=sin2[:, g0:g0 + 512])
                rot = dst[:, hl, g0:g0 + 512]
                nc.vector.tensor_add(out=rot, in0=src_q, in1=qsw[:])
                nc.vector.tensor_mul(out=rot, in0=rot, in1=rsf[:])

        def a_begin(b):
            st = state[b] = {}
            st["qT"] = p_qkv.tile([128, HPC, N], F16, tag="qT", name=f"qT{b}")
            st["kT"] = p_qkv.tile([128, HPC, N], F16, tag="kT", name=f"kT{b}")
            st["vtok"] = p_qkv.tile([128, 16, 256], F16, tag="v", name=f"v{b}")
            st["qraw"] = None
            st["ph1_pend"] = []
            st["ph2_pend"] = []

        def a_chunk_groups(b, ch):
            """QKV production for one 256-token chunk, as a generator that
            yields after each ~1.7us matmul group (6 groups per chunk)."""
            st = state[b]
            tok0 = b * N + ch * 256
            if ch % 2 == 0:
                st["qraw"] = p_qraw.tile([128, 4, 512], F16, tag="qraw",
                                         name=f"qraw{b}{ch}")
            qraw = st["qraw"]
            off = (ch % 2) * 256
            xts = []
            for half in range(2):
                xt = p_xt.tile([128, 8, 256], F16, tag="xt",
                               name=f"xt{b}{ch}{half}")
                src = xt_h.ap()[half * 1024:(half + 1) * 1024, tok0:tok0 + 256]
                if b == 0 and ch == 0:
                    # split so the first matmuls start after 0.5 MB, not 2 MB
                    nc.sync.dma_start(xt[:, 0:4, :],
                                      src[0:512].rearrange("(t p) j -> p t j", p=128))
                    nc.sync.dma_start(xt[:, 4:8, :],
                                      src[512:1024].rearrange("(t p) j -> p t j", p=128))
                else:
                    nc.sync.dma_start(xt[:], src.rearrange("(t p) j -> p t j", p=128))
                xts.append(xt)
            if b == 0 and ch == 0:
                nc.sync.dma_start(wqk[:, 0:8, 128:512],
                                  wqk_h.ap()[0:1024, 128:512]
                                  .rearrange("(t p) j -> p t j", p=128))
                load_late_consts()
            # q/k chan-tiles, one PSUM bank each, sequential over ct
            for ct in range(4):
                ps_qk = p_ps.tile([128, 256], F32, tag="qk", bufs=2,
                                  name=f"a{b}{ch}{ct}")
                for half in range(2):
                    for kt in range(8):
                        nc.tensor.matmul(
                            ps_qk[:], wqk[:, half * 8 + kt, ct * 128:(ct + 1) * 128],
                            xts[half][:, kt, :],
                            start=(half == 0 and kt == 0),
                            stop=(half == 1 and kt == 7))
                nc.vector.tensor_scalar_add(qraw[:, ct, off:off + 256],
                                            ps_qk[:], qkb[:, ct:ct + 1])
                # inject deferred norm work so its ACT/PE latency hides
                if ct == 1 and st["ph1_pend"]:
                    args = st["ph1_pend"].pop()
                    st["ph2_pend"].append((args[0], args[1], args[2],
                                           ph1(b, *args)))
                elif ct == 3 and st["ph2_pend"]:
                    ph2(b, *st["ph2_pend"].pop())
                yield
            for s in range(2):
                ps_v = p_ps.tile([128, 256], F32, tag="qk", bufs=2,
                                 name=f"av{b}{ch}{s}")
                for half in range(2):
                    for kt in range(8):
                        nc.tensor.matmul(
                            ps_v[:], xts[half][:, kt, s * 128:(s + 1) * 128],
                            wv[:, half * 8 + kt, :],
                            start=(half == 0 and kt == 0),
                            stop=(half == 1 and kt == 7))
                nc.vector.tensor_add(out=st["vtok"][:, ch * 2 + s, :],
                                     in0=ps_v[:], in1=vb[:])
                yield
            if ch % 2 == 1:
                st["ph1_pend"].append((qraw, (ch - 1) * 256, ch // 2))
            if ch == 7:
                # flush the last group's norm epilogue
                args = st["ph1_pend"].pop()
                ph2(b, args[0], args[1], args[2], ph1(b, *args))

        # ---------------- proj ----------------
        def c_quarters(b, tt, ring=None):
            """Partial proj + store for one 128-token tile; yields per oc.
            Copies ride the DVE except one per tile on ACT (ACT carries the
            exp cadence; DVE has the headroom)."""
            st = state[b]
            ao = st["ao"]
            for oc in range(4):
                ps_y = p_ps.tile([128, 512], F32, tag="qk", bufs=2,
                                 name=f"y{b}{tt}{oc}")
                for ct in range(2):
                    nc.tensor.matmul(ps_y[:], ao[:, ct, tt * 128:(tt + 1) * 128],
                                     wp[:, ct, oc * 512:(oc + 1) * 512],
                                     start=(ct == 0), stop=(ct == 1))
                yt = p_y.tile([128, 512], F16, tag="yt", name=f"yt{b}{tt}{oc}")
                if oc == 1 or (oc == 3 and tt % 2 == 1):
                    nc.scalar.copy(yt[:], ps_y[:])
                else:
                    nc.vector.tensor_copy(yt[:], ps_y[:])
                (ring or nc.sync).dma_start(
                    y_h.ap()[b * N + tt * 128:b * N + (tt + 1) * 128,
                             oc * 512:(oc + 1) * 512], yt[:])
                yield

        # ---------------- SDPA ----------------
        sdpa_pend = []   # deferred normalize tails

        def normalize(b, hl, qc, ps_av, rdf):
            st = state[b]
            q0 = qc * 512
            nc.vector.tensor_mul(out=st["ao"][:, hl, q0:q0 + 512],
                                 in0=ps_av[:], in1=rdf[:])

        def sdpa_unit(b, hl, qc, pull):
            """One (batch, head, 512-wide q chunk) SDPA unit.  `pull(n)` emits
            up to n filler work-groups (PE-dense) inside the kt loop."""
            st = state[b]
            if hl == 0 and qc == 0:
                st["ao"] = p_ao.tile([128, 2, N], F16, tag="ao", name=f"ao{b}")
            qT, kT, vtok = st["qT"], st["kT"], st["vtok"]
            q0 = qc * 512
            ex = p_exp.tile([128, 16, 512], F16, tag="ex", name=f"ex{b}{hl}{qc}")
            acc = p_acc.tile([128, 512], F16, tag="acc", name=f"acc{b}{hl}{qc}")
            ps_w = [None] * 8
            avst = {"ps": None}

            def qkpair(p):
                # 2-bank score tile; one exp instruction covers both k-tiles
                ps_w[p] = p_ps.tile([128, 2, 512], F32, tag="sw", bufs=2,
                                    name=f"s{b}{hl}{qc}{p}")
                for k in range(2):
                    kt = 2 * p + k
                    nc.tensor.matmul(ps_w[p][:, k, :],
                                     kT[:, hl, kt * 128:(kt + 1) * 128],
                                     qT[:, hl, q0:q0 + 512], start=True, stop=True)

            def exp_pair(p):
                nc.scalar.activation(ex[:, 2 * p:2 * p + 2, :], ps_w[p][:],
                                     mybir.ActivationFunctionType.Exp,
                                     bias=nb4[:], scale=SCALE)

            def av_acc_pair(p):
                j = 2 * p
                for kt in (j, j + 1):
                    nc.tensor.matmul(avst["ps"][:],
                                     vtok[:, kt, hl * 128:(hl + 1) * 128],
                                     ex[:, kt, :],
                                     start=(kt == 0), stop=(kt == 15))
                # denominator partials for kt 0-11 ride the DVE (in-place
                # accumulate); kt 12-15 are folded into the d-matmul below.
                # (Pool must stay broadcast-only — mixing op types thrashes
                # its ucode library, ~1us per switch.)
                if j == 0:
                    nc.vector.tensor_add(out=acc[:], in0=ex[:, 0, :],
                                         in1=ex[:, 1, :])
                elif j < 12:
                    nc.vector.tensor_add(out=acc[:], in0=acc[:], in1=ex[:, j, :])
                    nc.vector.tensor_add(out=acc[:], in0=acc[:],
                                         in1=ex[:, j + 1, :])

            for p in range(8):
                if p >= 1:
                    exp_pair(p - 1)
                qkpair(p)
                if p == 1:
                    if sdpa_pend:
                        normalize(*sdpa_pend.pop())
                    avst["ps"] = p_ps.tile([128, 512], F32, tag="av", bufs=2,
                                           name=f"o{b}{hl}{qc}")
                pull(1)
                if p >= 1:
                    av_acc_pair(p - 1)
            exp_pair(7)
            av_acc_pair(7)
            ps_av = avst["ps"]
            # denominator: partition-contract, invert, broadcast (Pool)
            ps_d = p_ps.tile([1, 512], F32, tag="qk", bufs=2,
                             name=f"d{b}{hl}{qc}")
            nc.tensor.matmul(ps_d[:], onecol[:], acc[:], start=True, stop=False)
            for kt in range(12, 16):
                nc.tensor.matmul(ps_d[:], onecol[:], ex[:, kt, :],
                                 start=False, stop=(kt == 15))
            ld = p_row.tile([1, 512], F32, tag="lrow", name=f"ld{b}{hl}{qc}")
            nc.scalar.activation(ld[:], ps_d[:],
                                 mybir.ActivationFunctionType.Ln)
            rd = p_row.tile([1, 512], F32, tag="rd", name=f"rd{b}{hl}{qc}")
            nc.scalar.activation(rd[:], ld[:],
                                 mybir.ActivationFunctionType.Exp, scale=-1.0)
            rdf = p_bcast.tile([128, 512], F32, tag="rdf", name=f"rdf{b}{hl}{qc}")
            nc.gpsimd.partition_broadcast(rdf[:], rd[:])
            sdpa_pend.append((b, hl, qc, ps_av, rdf))

        # ================= emission schedule =================
        # One global FIFO of PE-dense filler generators, pulled from inside
        # the SDPA kt loops: batch-1 QKV chunks first, then proj quarters of
        # each batch as its q-ranges get normalized.
        queue = []

        def pull(n):
            done = 0
            while queue and done < n:
                try:
                    next(queue[0])
                    done += 1
                except StopIteration:
                    queue.pop(0)
            return done

        a_begin(0)
        for ch in range(8):
            for _ in a_chunk_groups(0, ch):
                pass
        a_begin(1)
        queue.extend(a_chunk_groups(1, ch) for ch in range(8))

        # qc-major unit order: a q-range's second head finishes two units
        # after its first, so its proj quarters join the filler queue early
        for b in range(B):
            for qc in range(4):
                for hl in range(HPC):
                    if b == 0:
                        pulls = [1] * 8                      # chunk groups
                    else:
                        pulls = [2, 1] * 4                   # proj quarters
                    it = iter(pulls)
                    sdpa_unit(b, hl, qc, lambda n: pull(n * next(it)))
                if qc >= 1:
                    # proj for the q-range whose both heads are normalized
                    queue.extend(c_quarters(b, tt)
                                 for tt in range(4 * (qc - 1), 4 * qc))
                elif b == 1:
                    # batch-0 qc-3 became available at this phase's first pop
                    queue.extend(c_quarters(0, tt) for tt in range(12, 16))
        normalize(*sdpa_pend.pop())
        pull(10 ** 9)   # drain remaining proj work
        for tt in range(12, 16):
            for _ in c_quarters(1, tt, ring=nc.scalar):
                pass

    nc.compile()
    _CACHE["nc"] = nc
    return nc


def make_in_maps(x, rope, qkv_w, qkv_b, proj_w, q_norm_w, k_norm_w):
    """Host-side prep: transpose x, slice/scale weights per core."""
    x = np.asarray(x, np.float32)
    rope = np.asarray(rope, np.float32)
    qkv_w = np.asarray(qkv_w, np.float32)
    qkv_b = np.asarray(qkv_b, np.float32)
    proj_w = np.asarray(proj_w, np.float32)
    g_q = np.asarray(q_norm_w, np.float32)
    g_k = np.asarray(k_norm_w, np.float32)
    if np.any(g_q == 0) or np.any(g_k == 0):
        raise ValueError("zero rmsnorm weight not supported")

    xt = np.ascontiguousarray(x.reshape(TOK, C).T.astype(np.float16))  # [C, TOK]
    cos = np.cos(rope)                                        # [N, 64]
    sin = np.sin(rope)
    cos2 = np.ascontiguousarray(
        np.concatenate([cos, cos], axis=1).T.astype(np.float16))       # [128, N]
    sin2 = np.ascontiguousarray(
        np.concatenate([-sin, sin], axis=1).T.astype(np.float16))      # [128, N]
    invg2 = np.stack([1.0 / g_q ** 2, 1.0 / g_k ** 2], axis=1).astype(np.float16)
    onecol = np.ones((128, 1), np.float16)
    eps = np.full((1, 1), EPS, np.float32)
    nb4 = np.full((128, 1), ESHIFT, np.float32)

    in_maps = []
    for c in range(NCORES):
        hs = [HPC * c + hl for hl in range(HPC)]
        # chan-tiles: q_h0, q_h1, k_h0, k_h1 (g-scaled rows + bias)
        rows, biases = [], []
        for base, g in ((0, g_q), (C, g_k)):
            for h in hs:
                r0 = base + h * D
                rows.append(qkv_w[r0:r0 + D] * g[:, None])
                biases.append(qkv_b[r0:r0 + D] * g)
        wqk = np.ascontiguousarray(
            np.concatenate(rows, axis=0).T.astype(np.float16))           # [C, 512]
        qkb = np.stack(biases, axis=1)                                   # [128, 4]
        vrows = [qkv_w[2 * C + h * D:2 * C + (h + 1) * D] for h in hs]
        wv = np.ascontiguousarray(
            np.concatenate(vrows, axis=0).T.astype(np.float16))          # [C, 256]
        vbias = np.concatenate(
            [qkv_b[2 * C + h * D:2 * C + (h + 1) * D] for h in hs])      # [256]
        vb = np.broadcast_to(vbias, (128, 256)).astype(np.float32).copy()
        cols = np.concatenate([np.arange(h * D, (h + 1) * D) for h in hs])
        wpT = np.ascontiguousarray(proj_w[:, cols].T.astype(np.float16))  # [256, C]
        in_maps.append({
            "xt": xt, "wqk": wqk, "wv": wv, "wp": wpT,
            "cos2": cos2, "sin2": sin2, "qkb": qkb, "vb": vb,
            "invg2": invg2, "onecol": onecol, "eps": eps, "nb4": nb4,
        })
    return in_maps


def kernel(x, rope, qkv_w, qkv_b, proj_w, proj_b, q_norm_w, k_norm_w):
    nc = build_module()
    in_maps = make_in_maps(x, rope, qkv_w, qkv_b, proj_w, q_norm_w, k_norm_w)
    res = bass_utils.run_bass_kernel_spmd(nc, in_maps,
                                          core_ids=list(range(NCORES)), **RUN_KW)
    _CACHE["last_result"] = res
    y = np.zeros((TOK, C), np.float64)
    for c in range(NCORES):
        y += res.results[c]["y"].astype(np.float64)
    y += np.asarray(proj_b, np.float32).astype(np.float64)
    return y.astype(np.float32).reshape(B, N, C)
